# revision 2
# baseline (speedup 1.0000x reference)
"""Trainium2 Bass kernel for nn_LogicDense (difflogic dense layer), v5.

Math: out = c0 + ca*a + cb*b + cab*a*b, a = x[:, i0[j]], b = x[:, i1[j]].

v5 = hybrid-precision evolution of the v1 pipeline. Tensor-parallel over
out_dim (2048 rows/core, 16 chunks of 128). Chunks 0..M8-1 gather x as u8
(4 KiB rows), the rest as fp16 (8 KiB rows): per-core DMA drops from
48 MiB (v1) to 40 MiB (~118 us at ~354 GB/s), while engines keep slack
(DVE ~108, ACT ~90, Pool ~50), so the pipeline stays smooth/DMA-bound.

Per chunk c:
    u8 flavor:   t = ts(b_u8; cab/255^2, ca/255)   [DVE 2x_2p]
                 a_f = act(a_u8)                    [ACT]
                 h = act(b_u8; cb/255, c0)          [ACT]
    fp16 flavor: t = ts(b_f16; cab, ca)             [DVE 4x]
                 a_f = gathered directly
                 h = act(b_f16; cb, c0)             [ACT]
    both:        P = tt(t * a_f); out = tt(P + h)   [DVE], fp16 store.

Coeff scaling is folded per-chunk into the replicated GATE_COEFFS table
host-side; softmax + coeff collapse stays on device (v1 preamble).
"""

import os
import sys

import numpy as np

sys.path.insert(0, "/opt/trn_rl_repo")

BATCH = 4096
IN_DIM = 8192
OUT_DIM = 16384
N_CORES = 8
J_SHARD = OUT_DIM // N_CORES        # 2048 output rows per core
CHUNK = 128                         # output rows per compute chunk
N_CHUNKS = J_SHARD // CHUNK         # 16
GSIZE = 128                         # output rows per gather (1 chunk)
N_GATH = J_SHARD // GSIZE           # 16 gathers
NG = 6                              # gather buffer sets (1 chunk each)
NO = 4                              # output buffer sets
DVE_PRE = 14                        # DVE preamble instruction count
M8 = 8                              # chunks 0..M8-1 are u8-flavored

GATE_COEFFS = np.array([
    [0, 0, 0, 0], [0, 0, 0, 1], [0, 1, 0, -1], [0, 1, 0, 0],
    [0, 0, 1, -1], [0, 0, 1, 0], [0, 1, 1, -2], [0, 1, 1, -1],
    [1, -1, -1, 1], [1, -1, -1, 2], [1, 0, -1, 0], [1, 0, -1, 1],
    [1, -1, 0, 0], [1, -1, 0, 1], [1, 0, 0, -1], [1, 0, 0, 0],
], dtype=np.float32)

_CACHE = {}
LAST_RESULT = None


def is_u8(c):
    return c < M8


def _wrap_idx16(idx_pair):
    per = 2 * GSIZE // 16           # 16 columns per gather
    cols = []
    for g in range(N_GATH):
        merged = np.concatenate([idx_pair[0, g * GSIZE:(g + 1) * GSIZE],
                                 idx_pair[1, g * GSIZE:(g + 1) * GSIZE]])
        cols.append(merged.astype(np.int16).reshape(per, 16).T)
    blk = np.concatenate(cols, axis=1)
    return np.ascontiguousarray(np.tile(blk, (8, 1)))


def _build_program():
    import concourse.bacc as bacc
    import concourse.mybir as mybir
    from concourse.library_config import mlp
    from contextlib import ExitStack

    dt = mybir.dt
    AF = mybir.ActivationFunctionType
    MU, AD = mybir.AluOpType.mult, mybir.AluOpType.add

    nc = bacc.Bacc("TRN2", target_bir_lowering=False, debug=False,
                   num_swdge_queues=4)

    xt8 = nc.dram_tensor("xt8", [IN_DIM, BATCH], dt.uint8,
                         kind="ExternalInput")
    xtf = nc.dram_tensor("xtf", [IN_DIM, BATCH], dt.float16,
                         kind="ExternalInput")
    idx = nc.dram_tensor("idx", [128, 2 * (J_SHARD // 16)], dt.int16,
                         kind="ExternalInput")
    wgt = nc.dram_tensor("wgt", [128, N_CHUNKS * 16], dt.float32,
                         kind="ExternalInput")
    gcr = nc.dram_tensor("gcr", [128, 4 * N_CHUNKS * 16], dt.float32,
                         kind="ExternalInput")
    out = nc.dram_tensor("out", [J_SHARD, BATCH], dt.float16,
                         kind="ExternalOutput")

    W16 = N_CHUNKS * 16

    with ExitStack() as ctx:
        sb = lambda name, shape, dty: ctx.enter_context(
            nc.sbuf_tensor(name, shape, dty))
        sb_idx = sb("sb_idx", [128, 2 * (J_SHARD // 16)], dt.int16)
        sb_w = sb("sb_w", [128, W16], dt.float32)
        sb_gc = sb("sb_gc", [128, 4 * W16], dt.float32)
        sb_e = sb("sb_e", [128, W16], dt.float32)
        sb_scr = sb("sb_scr", [128, W16], dt.float32)
        sb_s = sb("sb_s", [128, N_CHUNKS], dt.float32)
        sb_r = sb("sb_r", [128, N_CHUNKS], dt.float32)
        sb_cc = sb("sb_cc", [128, 4 * N_CHUNKS], dt.float32)
        # gather buffers sized for the fp16 flavor (4 slots x 8 KiB);
        # u8 gathers use the first half, viewed as bytes.
        ab_bufs = [sb(f"ab{k}", [128, 2 * BATCH], dt.float16)
                   for k in range(NG)]
        tb = [sb(f"tb{k}", [128, BATCH], dt.float16) for k in range(2)]
        af = [sb(f"af{k}", [128, BATCH], dt.float16) for k in range(2)]
        hb = [sb(f"hb{k}", [128, BATCH], dt.float16) for k in range(2)]
        pf = [sb(f"pf{k}", [128, BATCH], dt.float16) for k in range(2)]
        o_bufs = [sb(f"o{k}", [128, BATCH], dt.float16) for k in range(NO)]

        def a_src(kg, c):
            """fp16 AP of the gathered a-rows for chunk c."""
            if is_u8(c):
                return None  # goes through af via ACT conv
            return ab_bufs[kg][:, 0:BATCH]

        def a_u8_src(kg, c):
            v = ab_bufs[kg].ap().bitcast(dt.uint8)
            return v[:, 0:BATCH]

        def b_src(kg, c):
            if is_u8(c):
                v = ab_bufs[kg].ap().bitcast(dt.uint8)
                return v[:, BATCH:2 * BATCH]
            return ab_bufs[kg][:, BATCH:2 * BATCH]

        # ---- static schedules -------------------------------------------
        # DVE: per chunk group c: [t(c), add(c-2), P(c-1)]; tails.
        ops_dve = []
        for c in range(N_CHUNKS):
            ops_dve.append(('t', c))
            if c >= 2:
                ops_dve.append(('add', c - 2))
            if c >= 1:
                ops_dve.append(('P', c - 1))
        ops_dve += [('P', N_CHUNKS - 1), ('add', N_CHUNKS - 2),
                    ('add', N_CHUNKS - 1)]
        dve_val = {op: DVE_PRE + n + 1 for n, op in enumerate(ops_dve)}

        # ACT: per chunk: [conv_a(c) if u8, h(c)]
        ops_act = []
        for c in range(N_CHUNKS):
            if is_u8(c):
                ops_act.append(('a', c))
            ops_act.append(('h', c))
        act_val = {op: n + 1 for n, op in enumerate(ops_act)}

        def act_count_through(c):
            n = 0
            for op, v in act_val.items():
                if op[1] <= c:
                    n = max(n, v)
            return n

        with (
            nc.Block() as block,
            nc.semaphore("s_pi") as s_pi,
            nc.semaphore("s_pw") as s_pw,
            nc.semaphore("s_pg") as s_pg,
            nc.semaphore("s_exp") as s_exp,
            nc.semaphore("s_g0") as s_g0,
            nc.semaphore("s_g1") as s_g1,
            nc.semaphore("s_g2") as s_g2,
            nc.semaphore("s_g3") as s_g3,
            nc.semaphore("s_g4") as s_g4,
            nc.semaphore("s_g5") as s_g5,
            nc.semaphore("s_st0") as s_st0,
            nc.semaphore("s_st1") as s_st1,
            nc.semaphore("s_st2") as s_st2,
            nc.semaphore("s_st3") as s_st3,
            nc.semaphore("s_act") as s_act,
            nc.semaphore("s_dve") as s_dve,
        ):
            s_g = [s_g0, s_g1, s_g2, s_g3, s_g4, s_g5]
            s_st = [s_st0, s_st1, s_st2, s_st3]

            def cseg(k, c):
                return sb_cc[:, 16 * k + c: 16 * k + c + 1]

            @block.sync
            def _(sync):
                sync.dma_start(sb_idx[:, :], idx[:, :]).then_inc(s_pi, 16)
                sync.dma_start(sb_w[:, :], wgt[:, :]).then_inc(s_pw, 16)
                sync.dma_start(sb_gc[:, :], gcr[:, :]).then_inc(s_pg, 16)
                for c in range(N_CHUNKS):
                    ko = c % NO
                    sync.wait_ge(s_dve, dve_val[('add', c)])
                    if c >= NO:
                        sync.wait_ge(s_st[ko], 16 * (c // NO))
                    sync.dma_start(out[c * CHUNK:(c + 1) * CHUNK, :],
                                   o_bufs[ko][:, :]).then_inc(s_st[ko], 16)
                for ko in range(NO):
                    n_st = (N_CHUNKS - 1 - ko) // NO + 1
                    sync.wait_ge(s_st[ko], 16 * n_st)

            @block.gpsimd
            def _(gp):
                gp.load_library(mlp)
                nreg = gp.alloc_register("nidx")
                gp.reg_mov(nreg, 2 * GSIZE)
                gp.wait_ge(s_pi, 16)
                per = 2 * GSIZE // 16
                for g in range(N_GATH):
                    kg = g % NG
                    if g >= NG:
                        cl = g - NG
                        gp.wait_ge(s_dve, dve_val[('P', cl)])
                        gp.wait_ge(s_act, act_count_through(cl))
                        gp.wait_ge(s_g[kg], 16 * (g // NG))
                    u8g = is_u8(g)
                    dst = (ab_bufs[kg].ap().bitcast(dt.uint8) if u8g
                           else ab_bufs[kg].ap())
                    dst = dst.rearrange("p (s b) -> p s b", b=BATCH)
                    dst = dst[:, 0:2, :]
                    gp.dma_gather(
                        dst, (xt8.ap() if u8g else xtf.ap()),
                        sb_idx[:, per * g: per * (g + 1)],
                        2 * GSIZE, nreg, BATCH,
                        queue_num=g % 4,
                    ).then_inc(s_g[kg], 16)

            @block.scalar
            def _(sc):
                sc.wait_ge(s_pw, 16)
                sc.activation(sb_e[:, :], sb_w[:, :], AF.Exp).then_inc(s_exp, 1)
                sc.wait_ge(s_dve, DVE_PRE)
                for op in ops_act:
                    kind, c = op
                    kg = c % NG
                    k2 = c % 2
                    sc.wait_ge(s_g[kg], 16 * (c // NG + 1))
                    if kind == 'a':
                        if c >= 2:      # af free once P(c-2) done
                            sc.wait_ge(s_dve, dve_val[('P', c - 2)])
                        sc.activation(af[k2][:, :], a_u8_src(kg, c),
                                      AF.Identity, bias=0.0, scale=1.0,
                                      ).then_inc(s_act, 1)
                    else:  # 'h'
                        if c >= 2:      # h buf free once add(c-2) done
                            sc.wait_ge(s_dve, dve_val[('add', c - 2)])
                        sc.activation(hb[k2][:, :], b_src(kg, c),
                                      AF.Identity,
                                      bias=cseg(0, c), scale=cseg(2, c),
                                      ).then_inc(s_act, 1)

            @block.vector
            def _(v):
                n = 0

                def step(ins):
                    nonlocal n
                    n += 1
                    ins.then_inc(s_dve, 1)

                v.wait_ge(s_exp, 1)
                v.wait_ge(s_pg, 16)
                e3 = sb_e[:, :].rearrange("p (c g) -> p c g", g=16)
                step(v.reduce_sum(sb_s[:, :], e3, axis=mybir.AxisListType.X))
                v.wait_ge(s_dve, n)
                step(v.reciprocal(sb_r[:, :], sb_s[:, :]))
                for kk in range(4):
                    if kk > 0:
                        v.wait_ge(s_dve, n)
                    step(v.tensor_mul(sb_scr[:, :], sb_e[:, :],
                                      sb_gc[:, kk * W16:(kk + 1) * W16]))
                    v.wait_ge(s_dve, n)
                    step(v.reduce_sum(
                        sb_cc[:, 16 * kk:16 * (kk + 1)],
                        sb_scr[:, :].rearrange("p (c g) -> p c g", g=16),
                        axis=mybir.AxisListType.X))
                v.wait_ge(s_dve, n)
                for kk in range(4):
                    step(v.tensor_mul(sb_cc[:, 16 * kk:16 * (kk + 1)],
                                      sb_cc[:, 16 * kk:16 * (kk + 1)],
                                      sb_r[:, :]))
                assert n == DVE_PRE

                for op in ops_dve:
                    kind, c = op
                    kg = c % NG
                    k2 = c % 2
                    if kind == 't':
                        v.wait_ge(s_g[kg], 16 * (c // NG + 1))
                        v.tensor_scalar(tb[k2][:, :], b_src(kg, c),
                                        cseg(3, c), cseg(1, c), MU, AD,
                                        ).then_inc(s_dve, 1)
                    elif kind == 'P':
                        asrc = a_src(kg, c)
                        if asrc is None:
                            v.wait_ge(s_act, act_val[('a', c)])
                            asrc = af[k2][:, :]
                        v.tensor_mul(pf[k2][:, :], tb[k2][:, :],
                                     asrc).then_inc(s_dve, 1)
                    else:  # add
                        v.wait_ge(s_act, act_val[('h', c)])
                        ko = c % NO
                        if c >= NO:
                            v.wait_ge(s_st[ko], 16 * (c // NO))
                        v.tensor_add(o_bufs[ko][:, :], pf[k2][:, :],
                                     hb[k2][:, :]).then_inc(s_dve, 1)

    nc.compile()
    return nc


def _get_program():
    if "nc" not in _CACHE:
        _CACHE["nc"] = _build_program()
    return _CACHE["nc"]


def kernel(x, weight, indices):
    global LAST_RESULT
    from concourse.bass_utils import run_bass_kernel_spmd

    x = np.asarray(x, dtype=np.float32)
    weight = np.asarray(weight, dtype=np.float32)
    indices = np.asarray(indices)

    nc = _get_program()

    xT = x.T
    xt8 = np.ascontiguousarray(
        np.clip(np.rint(xT * 255.0), 0, 255).astype(np.uint8))
    xtf = np.ascontiguousarray(xT.astype(np.float16))

    # per-chunk coeff scaling: u8 chunks fold the 1/255 quantization
    gc_rep = np.empty((4, N_CHUNKS, 16), dtype=np.float32)
    for c in range(N_CHUNKS):
        s = GATE_COEFFS.copy()
        if is_u8(c):
            s[:, 1] /= 255.0
            s[:, 2] /= 255.0
            s[:, 3] /= 255.0 ** 2
        gc_rep[:, c, :] = s.T
    gc_rep = gc_rep.reshape(1, -1)
    gc_rep = np.ascontiguousarray(
        np.broadcast_to(gc_rep, (128, 4 * N_CHUNKS * 16)).astype(np.float32))

    in_maps = []
    for c in range(N_CORES):
        j0 = c * J_SHARD
        idx_c = _wrap_idx16(indices[:, j0:j0 + J_SHARD])
        wsh = weight[j0:j0 + J_SHARD]
        w_wrapped = np.ascontiguousarray(
            wsh.reshape(N_CHUNKS, 128, 16).transpose(1, 0, 2)
            .reshape(128, N_CHUNKS * 16))
        in_maps.append({
            "xt8": xt8,
            "xtf": xtf,
            "idx": idx_c,
            "wgt": w_wrapped,
            "gcr": gc_rep,
        })

    trace = bool(os.environ.get("KERNEL_TRACE"))
    res = run_bass_kernel_spmd(nc, in_maps, core_ids=list(range(N_CORES)),
                               trace=trace)
    LAST_RESULT = res

    shards = [res.results[c]["out"] for c in range(N_CORES)]
    full = np.concatenate(shards, axis=0)
    return np.ascontiguousarray(full.T.astype(np.float32))
